# revision 56
# baseline (speedup 1.0000x reference)
"""Trainium2 Bass kernel for nn_MultiHeadAttention_61778809586301.

Head-sharded across 8 NeuronCores: core `a` computes output row-group `a`
(per the reference's faithful-TF recombination: head `a` across all 8
batches, concatenated batch-major along channels, then Wo+relu).

Design:
  - scores computed TRANSPOSED (S^T = K^T blocks vs Q panels) so the P^T
    the PV matmul needs comes straight out of exp() — no P transposes.
  - no max-subtraction softmax; the PV matmul also emits the softmax
    denominator via an extra ones column; normalization happens on the
    small O^T via an fp32r broadcast + approx reciprocal.
  - KEY+QUERY COMPACTION: each core attends with one key_mask row and one
    query_mask row (faithful-TF quirk).  The host compacts away masked
    keys AND masked queries per core (~2x fewer of each), uploads
    per-core compacted xt_q/xt_k/xt_v and a compacted-coordinate
    visibility mask (causal+pad) applied after exp.  Panel bounds are
    fixed at build time from the min visible-query position across cores.
    Dropped query rows are exactly zero in the reference (relu * qm), so
    the host scatters device rows into a zeros array afterwards.
  - fully-masked-row semantics handled by host-precomputed corrections.
"""
import sys

if "/opt/trn_rl_repo" not in sys.path:
    sys.path.insert(0, "/opt/trn_rl_repo")

import numpy as np

B, S, D, H, DH = 8, 1024, 512, 8, 64
NPAIR = 4          # batch pairs (p, p+4)
KO = D // 128      # 4 contraction chunks of 128

_CACHE: dict = {}
RUN_KWARGS: dict = {}   # extra kwargs for run_bass_kernel_spmd (e.g. trace)
LAST_RESULT = None      # BassKernelResults of the most recent kernel() call


def _chunks_for(c0, nq):
    """Split compacted-q columns [c0, nq) at the 512 PSUM-bank boundary."""
    if c0 < 512 < nq:
        return [(0, 512 - c0), (512 - c0, nq - c0)]
    return [(0, nq - c0)]


def _build(nkc, sb, nq):
    import concourse.mybir as mybir
    import concourse.tile as tile
    from concourse import bacc
    from concourse.masks import make_identity

    nblkc = nkc // 128
    nqblk = nq // 128
    pw = [nq - 128 * sb[j] for j in range(nblkc)]   # panel widths
    off = np.concatenate([[0], np.cumsum(pw)]).astype(int)
    sumw = int(off[-1])
    # last panel whose chunks touch PSUM bank 0 (compacted-q col < 512)
    lb0 = max(j for j in range(nblkc) if 128 * sb[j] < 512)
    qch = _chunks_for(0, nq)    # [(0,512),(512,nq)] or [(0,nq)]

    f32 = mybir.dt.float32
    bf16 = mybir.dt.bfloat16
    nc = bacc.Bacc(
        "TRN2",
        target_bir_lowering=False,
        debug=False,
        enable_asserts=False,
        num_devices=H,
    )

    xt_q = nc.dram_tensor("xt_q", [D, B * nq], bf16, kind="ExternalInput")
    xt_k = nc.dram_tensor("xt_k", [D, B * nkc], bf16, kind="ExternalInput")
    xt_v = nc.dram_tensor("xt_v", [D, B * nkc], bf16, kind="ExternalInput")
    wq_d = nc.dram_tensor("wq", [D, DH], bf16, kind="ExternalInput")
    wk_d = nc.dram_tensor("wk", [D, DH], bf16, kind="ExternalInput")
    wv_d = nc.dram_tensor("wv", [D, DH], bf16, kind="ExternalInput")
    wo_d = nc.dram_tensor("wo_p", [NPAIR, 128, D], bf16, kind="ExternalInput")
    msk_d = nc.dram_tensor("maskc", [128, sumw], bf16, kind="ExternalInput")
    kmc_d = nc.dram_tensor("kmc", [128, nblkc], bf16, kind="ExternalInput")
    flg_d = nc.dram_tensor("flg", [1, nq], bf16, kind="ExternalInput")
    corr_d = nc.dram_tensor("corrT", [128, NPAIR, 128], f32,
                            kind="ExternalInput")
    out_d = nc.dram_tensor("out", [nq, D], f32, kind="ExternalOutput")

    kvch = [(0, min(512, nkc))] + ([(512, nkc)] if nkc > 512 else [])

    with tile.TileContext(nc) as tc:
        with (
            tc.tile_pool(name="fixed", bufs=1) as fixed,
            tc.tile_pool(name="stage", bufs=3) as stage,
            tc.tile_pool(name="proj", bufs=2) as proj,
            tc.tile_pool(name="ptp", bufs=5) as ptp,
            tc.tile_pool(name="rowp", bufs=2) as rowp,
            tc.tile_pool(name="outp", bufs=3) as outp,
            tc.tile_pool(name="psS", bufs=3, space="PSUM") as psS,
            tc.tile_pool(name="psOT", bufs=2, space="PSUM") as psOT,
            tc.tile_pool(name="psT", bufs=1, space="PSUM") as psT,
        ):
            # ---- constants / weights ----
            ident = fixed.tile([128, 128], f32, tag="ident")
            make_identity(nc, ident[:])
            ident_bf = fixed.tile([128, 128], bf16, tag="identbf")
            nc.gpsimd.tensor_copy(ident_bf[:], ident[:])

            wq_sb = fixed.tile([128, KO, DH], bf16, tag="wq")
            wk_sb = fixed.tile([128, KO, DH], bf16, tag="wk")
            wv_sb = fixed.tile([128, KO, DH], bf16, tag="wv")
            nc.scalar.dma_start(wq_sb[:], wq_d.rearrange("(ko ki) m -> ki ko m", ki=128))
            nc.scalar.dma_start(wk_sb[:], wk_d.rearrange("(ko ki) m -> ki ko m", ki=128))
            nc.scalar.dma_start(wv_sb[:], wv_d.rearrange("(ko ki) m -> ki ko m", ki=128))

            msk_sb = fixed.tile([128, sumw], bf16, tag="msk")
            nc.gpsimd.dma_start(msk_sb[:], msk_d[:, :])
            kmc_sb = fixed.tile([128, nblkc], bf16, tag="kmc")
            nc.gpsimd.dma_start(kmc_sb[:], kmc_d[:, :])
            flg_sb = fixed.tile([1, nq], bf16, tag="flg")
            nc.gpsimd.dma_start(flg_sb[:], flg_d[:, :])
            # wo/corr are consumed late; loads emitted after pair-0 staging
            wo_sb = fixed.tile([128, NPAIR, D], bf16, tag="wo")
            corr_sb = fixed.tile([128, NPAIR, 128], f32, tag="corr")

            # ones rows for the rcp broadcast (K=2: sum row + flag row)
            f32r = mybir.dt.float32r
            ones64f = fixed.tile([2, DH], f32, tag="ones64f")
            nc.vector.memset(ones64f[:], 1.0)
            ones64 = fixed.tile([2, DH], f32r, tag="ones64")
            nc.scalar.copy(ones64[:], ones64f[:])

            # persistent per-half sum-row tiles; row 1 preloaded with the
            # flag-row indicator so the broadcast matmul adds it for free
            flgr = fixed.tile([1, nq], f32r, tag="flgr")
            nc.scalar.copy(flgr[:], flg_sb[:])
            sr_tiles = []
            for gg in range(2):
                srt = fixed.tile([2, nq], f32r, tag=f"srt{gg}",
                                 name=f"srt{gg}")
                nc.scalar.dma_start(srt[1:2, :], flgr[0:1, :])
                sr_tiles.append(srt)

            # persistent attention outputs, transposed
            ot_sb = [
                fixed.tile([128, nq], bf16, tag=f"ot{p}", name=f"ot{p}")
                for p in range(NPAIR)
            ]

            def emit_proj(p):
                """q/k/v projections for pair p (all on compacted streams)."""
                qkv = []
                for name, xt, wid, ch in (
                    ("q", xt_q, nq, qch), ("k", xt_k, nkc, kvch),
                    ("v", xt_v, nkc, kvch),
                ):
                    pair_t = proj.tile([128, wid], bf16, tag=f"{name}T",
                                       name=f"{name}T{p}")
                    sts = []
                    for g in range(2):
                        c = p + 4 * g
                        st = stage.tile(
                            [128, KO, wid], bf16,
                            tag="xsq" if name == "q" else "xskv",
                            bufs=4 if name == "q" else 8,
                            name=f"st{p}{name}{g}",
                        )
                        # q staging issues on sync, k/v on scalar: the two
                        # queues issue in parallel (DIRECT2D issue slices
                        # are ~1.2us each and serialize per queue)
                        dq = nc.sync if name == "q" else nc.scalar
                        dq.dma_start(
                            st[:],
                            xt[:, c * wid:(c + 1) * wid].rearrange(
                                "(ko ki) s -> ki ko s", ki=128
                            ),
                        )
                        sts.append(st)
                    w_sb = {"q": wq_sb, "k": wk_sb, "v": wv_sb}[name]
                    for (c0, c1) in ch:
                        ps = psS.tile([128, 512], f32, tag="ps",
                                      name=f"psp{p}{name}{c0}")
                        for ko in range(KO):
                            for g in range(2):
                                nc.tensor.matmul(
                                    ps[64 * g:64 * (g + 1), 0:c1 - c0],
                                    lhsT=w_sb[:, ko, :],
                                    rhs=sts[g][:, ko, c0:c1],
                                    start=(ko == 0),
                                    stop=(ko == KO - 1),
                                )
                        nc.vector.tensor_copy(
                            pair_t[:, c0:c1], ps[:, 0:c1 - c0],
                        )
                    qkv.append(pair_t)
                qT, kT, vm = qkv

                # V to natural layout via PE transpose, both halves at once;
                # vnat[:, j, g, 64] = 1 for real (non-pad) compacted keys
                vnat = proj.tile([128, nblkc, 2, DH + 1], bf16, tag="vnat",
                                 name=f"vnat{p}")
                for j in range(nblkc):
                    pst = psT.tile([128, 128], bf16, tag="pst",
                                   name=f"pst{p}{j}")
                    nc.tensor.transpose(
                        pst[:], vm[:, 128 * j:128 * (j + 1)], ident_bf[:]
                    )
                    nc.vector.tensor_copy(vnat[:, j, :, 0:DH], pst[:])
                nc.gpsimd.tensor_copy(
                    vnat[:, :, :, DH],
                    kmc_sb[:, :, None].to_broadcast((128, nblkc, 2)),
                )
                return qT, kT, vnat

            def emit_attn(p, tiles):
                qT, kT, vnat = tiles
                pos = {
                    g: psOT.tile([DH + 1, nq], f32, tag="psot",
                                 name=f"po{p}{g}")
                    for g in range(2)
                }

                def emit_panel(j, g):
                    """scores^T panel j + exp + visibility mask -> pt."""
                    gs = slice(64 * g, 64 * (g + 1))
                    c0 = 128 * sb[j]
                    chunks = _chunks_for(c0, nq)
                    pt = ptp.tile([128, nq - c0], bf16, tag="pt",
                                  name=f"pt{p}{g}{j}")
                    for (lo, hi) in chunks:
                        ss = psS.tile([128, 512], f32, tag="ps",
                                      name=f"ss{p}{g}{j}{lo}")
                        nc.tensor.matmul(
                            ss[:, 0:hi - lo],
                            lhsT=kT[gs, 128 * j:128 * (j + 1)],
                            rhs=qT[gs, c0 + lo:c0 + hi],
                            start=True,
                            stop=True,
                        )
                        nc.scalar.activation(
                            pt[:, lo:hi],
                            ss[:, 0:hi - lo],
                            mybir.ActivationFunctionType.Exp,
                            bias=0.0,
                            scale=1.0,
                        )
                        # visibility mask: causal (compacted coords) + pads;
                        # split across DVE/Pool
                        eng = nc.vector if (j + g) % 2 else nc.gpsimd
                        eng.tensor_tensor(
                            pt[:, lo:hi],
                            pt[:, lo:hi],
                            msk_sb[:, int(off[j]) + lo:int(off[j]) + hi],
                            mybir.AluOpType.mult,
                        )
                    return pt, chunks

                def emit_pv(j, g, pt, chunks):
                    # PV accumulate: po[:, q] += vnat_j^T @ pt
                    c0 = 128 * sb[j]
                    for (lo, hi) in chunks:
                        stop = (j == lb0 and c0 + lo < 512) or (
                            j == nblkc - 1 and c0 + lo >= 512
                        )
                        nc.tensor.matmul(
                            pos[g][:, c0 + lo:c0 + hi],
                            lhsT=vnat[:, j, g, :],
                            rhs=pt[:, lo:hi],
                            start=(j == 0),
                            stop=bool(stop),
                            skip_group_check=True,
                        )

                # g-interleaved panels with a one-step j pipeline
                prev = [emit_panel(0, 0), emit_panel(0, 1)]
                for j in range(1, nblkc):
                    cur = [emit_panel(j, 0), emit_panel(j, 1)]
                    for g in range(2):
                        emit_pv(j - 1, g, *prev[g])
                    prev = cur
                for g in range(2):
                    emit_pv(nblkc - 1, g, *prev[g])
                    nc.scalar.copy(sr_tiles[g][0:1, :],
                                   pos[g][DH:DH + 1, :])
                # tails: bcast(sum+flag), approx reciprocal, multiply
                for g in range(2):
                    gs = slice(64 * g, 64 * (g + 1))
                    po = pos[g]
                    for (c0, c1) in qch:
                        cols = slice(c0, c1)
                        bc = psS.tile([128, 512], f32, tag="ps",
                                      name=f"bc{p}{g}{c0}")
                        nc.tensor.matmul(
                            bc[0:DH, 0:c1 - c0],
                            lhsT=ones64[:, :],
                            rhs=sr_tiles[g][:, cols],
                            start=True,
                            stop=True,
                        )
                        rcpb = rowp.tile([DH, 512], f32, tag="bcs",
                                         name=f"rcpb{p}{g}{c0}")
                        nc.vector.reciprocal_approx_fast(
                            rcpb[:, 0:c1 - c0], bc[0:DH, 0:c1 - c0]
                        )
                        nc.vector.tensor_tensor(
                            ot_sb[p][gs, cols],
                            po[0:DH, cols],
                            rcpb[:, 0:c1 - c0],
                            mybir.AluOpType.mult,
                        )
                    nc.vector.tensor_tensor(
                        ot_sb[p][gs, 0:128],
                        ot_sb[p][gs, 0:128],
                        corr_sb[gs, p, :],
                        mybir.AluOpType.add,
                    )

            # ---- software-pipelined emission: proj one pair ahead ----
            tiles = emit_proj(0)
            nc.sync.dma_start(wo_sb[:], wo_d.rearrange("p ki n -> ki p n"))
            nc.sync.dma_start(corr_sb[:], corr_d[:, :, :])
            nxt = emit_proj(1)
            emit_attn(0, tiles)
            tiles, nxt = nxt, emit_proj(2)
            emit_attn(1, tiles)
            tiles, nxt = nxt, emit_proj(3)
            emit_attn(2, tiles)
            emit_attn(3, nxt)

            # ---- final projection + relu (qm already applied by
            # compaction: dropped rows are host-scattered zeros) ----
            for i in range(nqblk):
                ps = psS.tile([128, 512], f32, tag="ps", name=f"psf{i}")
                for p in range(NPAIR):
                    nc.tensor.matmul(
                        ps[:],
                        lhsT=ot_sb[p][:, 128 * i:128 * (i + 1)],
                        rhs=wo_sb[:, p, :],
                        start=(p == 0),
                        stop=(p == NPAIR - 1),
                    )
                o_sb = outp.tile([128, D], f32, tag="osb", name=f"osb{i}")
                nc.scalar.activation(
                    o_sb[:],
                    ps[:],
                    mybir.ActivationFunctionType.Relu,
                    bias=0.0,
                    scale=1.0,
                )
                nc.sync.dma_start(out_d[128 * i:128 * (i + 1), :], o_sb[:])

    nc.compile()
    return nc


def _get_nc(nkc, sb, nq):
    key = (nkc, tuple(sb), nq)
    if key not in _CACHE:
        _CACHE[key] = _build(nkc, sb, nq)
    return _CACHE[key]


def _host_prep(query, key, value, query_mask, key_mask, Wq, Wk, Wv, Wo):
    """Per-core input maps + shared compaction geometry."""
    inv = np.float32(1.0) / np.sqrt(np.float32(D))

    import ml_dtypes

    bfl = ml_dtypes.bfloat16

    def tfeat(x):  # (B,S,D) -> feature-major (D, B*S), contiguous bf16
        return np.ascontiguousarray(
            x.reshape(B * S, D).astype(np.float32, copy=False).T
        ).astype(bfl)

    xq, xk, xv = tfeat(query), tfeat(key), tfeat(value)
    kmf = key_mask.astype(np.float32)
    qmf = query_mask.astype(np.float32)
    Wqf = Wq.astype(np.float32, copy=False)
    Wkf = Wk.astype(np.float32, copy=False)
    Wvf = Wv.astype(np.float32, copy=False)
    Wof = Wo.astype(np.float32, copy=False)

    wo_p = np.stack(
        [
            np.concatenate(
                [Wof[p * DH:(p + 1) * DH, :], Wof[(p + 4) * DH:(p + 5) * DH, :]],
                axis=0,
            )
            for p in range(NPAIR)
        ]
    ).astype(bfl)  # (4, 128, 512)

    # ---- compaction geometry (shared across cores at build time) ----
    idxk = [np.nonzero(kmf[a])[0] for a in range(H)]
    idxq = [np.nonzero(qmf[a])[0] for a in range(H)]
    nkc = 128 * int(np.ceil(max(len(i) for i in idxk) / 128.0))
    nq = 128 * int(np.ceil(max(len(i) for i in idxq) / 128.0))
    nblkc = nkc // 128
    sb = []
    for jp in range(nblkc):
        starts = []
        for a in range(H):
            if len(idxk[a]) > 128 * jp:
                pos = int(idxk[a][128 * jp])
                starts.append(int(np.searchsorted(idxq[a], pos)))
        sb.append(min(starts) // 128 if starts else 0)
    assert sb[0] == 0, "first compacted key block must start at q block 0"
    pw = [nq - 128 * sbj for sbj in sb]
    off = np.concatenate([[0], np.cumsum(pw)]).astype(int)
    sumw = int(off[-1])

    in_maps = []
    for a in range(H):
        km = kmf[a]
        ik, iq = idxk[a], idxq[a]
        n_k, n_q = len(ik), len(iq)
        # flag rows: every visible key masked (faithful-TF uniform tie case)
        cs = np.cumsum(km)
        flg_full = (cs == 0).astype(np.float32)      # (S,), original coords
        corrT = np.zeros((128, NPAIR, 128), np.float32)
        kept_flag = [cq for cq in range(n_q) if flg_full[iq[cq]]]
        if kept_flag:
            assert max(kept_flag) < 128, "flag rows beyond block 0"
            wv_a = Wvf[:, a * DH:(a + 1) * DH]       # (512, 64)
            tail_cnt = km.sum()
            for p in range(NPAIR):
                for g in range(2):
                    c = p + 4 * g
                    vfull = value[c].astype(np.float32)      # (S, 512)
                    mtot = (km[:, None] * vfull).sum(axis=0)  # (512,)
                    for cq in kept_flag:
                        sq = int(iq[cq])
                        pre = vfull[:sq + 1].sum(axis=0)
                        count = (sq + 1) + tail_cnt
                        corrT[64 * g:64 * (g + 1), p, cq] = (
                            (pre + mtot) @ wv_a
                        ) / np.float32(count)

        # per-core compacted streams (pads stay zero)
        xq_c = np.zeros((D, B * nq), bfl)
        xk_c = np.zeros((D, B * nkc), bfl)
        xv_c = np.zeros((D, B * nkc), bfl)
        for c in range(B):
            xq_c[:, c * nq:c * nq + n_q] = xq[:, c * S + iq]
            xk_c[:, c * nkc:c * nkc + n_k] = xk[:, c * S + ik]
            xv_c[:, c * nkc:c * nkc + n_k] = xv[:, c * S + ik]

        # visibility mask in compacted coords: key r of block jp visible to
        # compacted query cq iff ik[...] <= iq[cq]; pads never visible
        maskc = np.zeros((128, sumw), np.float32)
        for jp in range(nblkc):
            base = 128 * sb[jp]
            for r in range(128):
                ki = 128 * jp + r
                if ki < n_k:
                    cq0 = int(np.searchsorted(iq, int(ik[ki])))
                    s0 = max(0, cq0 - base)
                    maskc[r, int(off[jp]) + s0:int(off[jp + 1])] = 1.0
        kmc = np.zeros((128, nblkc), np.float32)
        for jp in range(nblkc):
            kmc[:, jp] = (128 * jp + np.arange(128) < n_k)
        # flag indicator in compacted coords; pads flagged too (keeps the
        # reciprocal away from 0)
        flg_c = np.ones(nq, np.float32)
        flg_c[:n_q] = flg_full[iq]

        in_maps.append(
            {
                "xt_q": xq_c,
                "xt_k": xk_c,
                "xt_v": xv_c,
                "wq": np.ascontiguousarray(
                    Wqf[:, a * DH:(a + 1) * DH] * inv
                ).astype(bfl),
                "wk": np.ascontiguousarray(
                    Wkf[:, a * DH:(a + 1) * DH]
                ).astype(bfl),
                "wv": np.ascontiguousarray(
                    Wvf[:, a * DH:(a + 1) * DH]
                ).astype(bfl),
                "wo_p": wo_p,
                "maskc": maskc.astype(bfl),
                "kmc": kmc.astype(bfl),
                "flg": np.ascontiguousarray(flg_c[None, :]).astype(bfl),
                "corrT": corrT,
            }
        )
    return in_maps, nkc, sb, nq, idxq


def kernel(**inputs) -> np.ndarray:
    from concourse.bass_utils import run_bass_kernel_spmd

    in_maps, nkc, sb, nq, idxq = _host_prep(
        np.asarray(inputs["query"]),
        np.asarray(inputs["key"]),
        np.asarray(inputs["value"]),
        np.asarray(inputs["query_mask"]),
        np.asarray(inputs["key_mask"]),
        np.asarray(inputs["Wq"]),
        np.asarray(inputs["Wk"]),
        np.asarray(inputs["Wv"]),
        np.asarray(inputs["Wo"]),
    )
    nc = _get_nc(nkc, sb, nq)
    res = run_bass_kernel_spmd(nc, in_maps, core_ids=list(range(H)), **RUN_KWARGS)
    global LAST_RESULT
    LAST_RESULT = res
    full = np.zeros((H, S, D), np.float32)
    for a in range(H):
        n_q = len(idxq[a])
        full[a][idxq[a]] = res.results[a]["out"][:n_q]
    return full


# revision 57
# speedup vs baseline: 1.0008x; 1.0008x over previous
"""Trainium2 Bass kernel for nn_MultiHeadAttention_61778809586301.

Head-sharded across 8 NeuronCores: core `a` computes output row-group `a`
(per the reference's faithful-TF recombination: head `a` across all 8
batches, concatenated batch-major along channels, then Wo+relu).

Design:
  - scores computed TRANSPOSED (S^T = K^T blocks vs Q panels) so the P^T
    the PV matmul needs comes straight out of exp() — no P transposes.
  - no max-subtraction softmax; the PV matmul also emits the softmax
    denominator via an extra ones column; normalization happens on the
    small O^T via an fp32r broadcast + approx reciprocal.
  - KEY+QUERY COMPACTION: each core attends with one key_mask row and one
    query_mask row (faithful-TF quirk).  The host compacts away masked
    keys AND masked queries per core (~2x fewer of each), uploads
    per-core compacted xt_q/xt_k/xt_v and a compacted-coordinate
    visibility mask (causal+pad) applied after exp.  Panel bounds are
    fixed at build time from the min visible-query position across cores.
    Dropped query rows are exactly zero in the reference (relu * qm), so
    the host scatters device rows into a zeros array afterwards.
  - fully-masked-row semantics handled by host-precomputed corrections.
"""
import sys

if "/opt/trn_rl_repo" not in sys.path:
    sys.path.insert(0, "/opt/trn_rl_repo")

import numpy as np

B, S, D, H, DH = 8, 1024, 512, 8, 64
NPAIR = 4          # batch pairs (p, p+4)
KO = D // 128      # 4 contraction chunks of 128

_CACHE: dict = {}
RUN_KWARGS: dict = {}   # extra kwargs for run_bass_kernel_spmd (e.g. trace)
LAST_RESULT = None      # BassKernelResults of the most recent kernel() call


def _chunks_for(c0, nq):
    """Split compacted-q columns [c0, nq) at the 512 PSUM-bank boundary."""
    if c0 < 512 < nq:
        return [(0, 512 - c0), (512 - c0, nq - c0)]
    return [(0, nq - c0)]


def _build(nkc, sb, nq):
    import concourse.mybir as mybir
    import concourse.tile as tile
    from concourse import bacc
    from concourse.masks import make_identity

    nblkc = nkc // 128
    nqblk = nq // 128
    pw = [nq - 128 * sb[j] for j in range(nblkc)]   # panel widths
    off = np.concatenate([[0], np.cumsum(pw)]).astype(int)
    sumw = int(off[-1])
    # last panel whose chunks touch PSUM bank 0 (compacted-q col < 512)
    lb0 = max(j for j in range(nblkc) if 128 * sb[j] < 512)
    qch = _chunks_for(0, nq)    # [(0,512),(512,nq)] or [(0,nq)]

    f32 = mybir.dt.float32
    bf16 = mybir.dt.bfloat16
    nc = bacc.Bacc(
        "TRN2",
        target_bir_lowering=False,
        debug=False,
        enable_asserts=False,
        num_devices=H,
    )

    xt_q = nc.dram_tensor("xt_q", [D, B * nq], bf16, kind="ExternalInput")
    xt_k = nc.dram_tensor("xt_k", [D, B * nkc], bf16, kind="ExternalInput")
    xt_v = nc.dram_tensor("xt_v", [D, B * nkc], bf16, kind="ExternalInput")
    wq_d = nc.dram_tensor("wq", [D, DH], bf16, kind="ExternalInput")
    wk_d = nc.dram_tensor("wk", [D, DH], bf16, kind="ExternalInput")
    wv_d = nc.dram_tensor("wv", [D, DH], bf16, kind="ExternalInput")
    wo_d = nc.dram_tensor("wo_p", [NPAIR, 128, D], bf16, kind="ExternalInput")
    msk_d = nc.dram_tensor("maskc", [128, sumw], bf16, kind="ExternalInput")
    kmc_d = nc.dram_tensor("kmc", [128, nblkc], bf16, kind="ExternalInput")
    flg_d = nc.dram_tensor("flg", [1, nq], bf16, kind="ExternalInput")
    corr_d = nc.dram_tensor("corrT", [128, NPAIR, 128], f32,
                            kind="ExternalInput")
    out_d = nc.dram_tensor("out", [nq, D], f32, kind="ExternalOutput")

    kvch = [(0, min(512, nkc))] + ([(512, nkc)] if nkc > 512 else [])

    with tile.TileContext(nc) as tc:
        with (
            tc.tile_pool(name="fixed", bufs=1) as fixed,
            tc.tile_pool(name="stage", bufs=3) as stage,
            tc.tile_pool(name="proj", bufs=2) as proj,
            tc.tile_pool(name="ptp", bufs=5) as ptp,
            tc.tile_pool(name="rowp", bufs=2) as rowp,
            tc.tile_pool(name="outp", bufs=3) as outp,
            tc.tile_pool(name="psS", bufs=3, space="PSUM") as psS,
            tc.tile_pool(name="psOT", bufs=2, space="PSUM") as psOT,
            tc.tile_pool(name="psT", bufs=1, space="PSUM") as psT,
        ):
            # ---- constants / weights ----
            ident = fixed.tile([128, 128], f32, tag="ident")
            make_identity(nc, ident[:])
            ident_bf = fixed.tile([128, 128], bf16, tag="identbf")
            nc.gpsimd.tensor_copy(ident_bf[:], ident[:])

            wq_sb = fixed.tile([128, KO, DH], bf16, tag="wq")
            wk_sb = fixed.tile([128, KO, DH], bf16, tag="wk")
            wv_sb = fixed.tile([128, KO, DH], bf16, tag="wv")
            nc.scalar.dma_start(wq_sb[:], wq_d.rearrange("(ko ki) m -> ki ko m", ki=128))
            nc.scalar.dma_start(wk_sb[:], wk_d.rearrange("(ko ki) m -> ki ko m", ki=128))
            nc.scalar.dma_start(wv_sb[:], wv_d.rearrange("(ko ki) m -> ki ko m", ki=128))

            msk_sb = fixed.tile([128, sumw], bf16, tag="msk")
            nc.gpsimd.dma_start(msk_sb[:], msk_d[:, :])
            kmc_sb = fixed.tile([128, nblkc], bf16, tag="kmc")
            nc.gpsimd.dma_start(kmc_sb[:], kmc_d[:, :])
            flg_sb = fixed.tile([1, nq], bf16, tag="flg")
            nc.gpsimd.dma_start(flg_sb[:], flg_d[:, :])
            # wo/corr are consumed late; loads emitted after pair-0 staging
            wo_sb = fixed.tile([128, NPAIR, D], bf16, tag="wo")
            corr_sb = fixed.tile([128, NPAIR, 128], f32, tag="corr")

            # ones rows for the rcp broadcast (K=2: sum row + flag row)
            f32r = mybir.dt.float32r
            ones64f = fixed.tile([2, DH], f32, tag="ones64f")
            nc.vector.memset(ones64f[:], 1.0)
            ones64 = fixed.tile([2, DH], f32r, tag="ones64")
            nc.scalar.copy(ones64[:], ones64f[:])

            # persistent per-half sum-row tiles; row 1 preloaded with the
            # flag-row indicator so the broadcast matmul adds it for free
            flgr = fixed.tile([1, nq], f32r, tag="flgr")
            nc.scalar.copy(flgr[:], flg_sb[:])
            sr_tiles = []
            for gg in range(2):
                srt = fixed.tile([2, nq], f32r, tag=f"srt{gg}",
                                 name=f"srt{gg}")
                nc.scalar.dma_start(srt[1:2, :], flgr[0:1, :])
                sr_tiles.append(srt)

            # persistent attention outputs, transposed
            ot_sb = [
                fixed.tile([128, nq], bf16, tag=f"ot{p}", name=f"ot{p}")
                for p in range(NPAIR)
            ]

            def emit_proj(p):
                """q/k/v projections for pair p (all on compacted streams)."""
                qkv = []
                for name, xt, wid, ch in (
                    ("q", xt_q, nq, qch), ("k", xt_k, nkc, kvch),
                    ("v", xt_v, nkc, kvch),
                ):
                    pair_t = proj.tile([128, wid], bf16, tag=f"{name}T",
                                       name=f"{name}T{p}")
                    sts = []
                    for g in range(2):
                        c = p + 4 * g
                        st = stage.tile(
                            [128, KO, wid], bf16,
                            tag="xsq" if name == "q" else "xskv",
                            bufs=4 if name == "q" else 8,
                            name=f"st{p}{name}{g}",
                        )
                        nc.sync.dma_start(
                            st[:],
                            xt[:, c * wid:(c + 1) * wid].rearrange(
                                "(ko ki) s -> ki ko s", ki=128
                            ),
                        )
                        sts.append(st)
                    w_sb = {"q": wq_sb, "k": wk_sb, "v": wv_sb}[name]
                    for (c0, c1) in ch:
                        ps = psS.tile([128, 512], f32, tag="ps",
                                      name=f"psp{p}{name}{c0}")
                        for ko in range(KO):
                            for g in range(2):
                                nc.tensor.matmul(
                                    ps[64 * g:64 * (g + 1), 0:c1 - c0],
                                    lhsT=w_sb[:, ko, :],
                                    rhs=sts[g][:, ko, c0:c1],
                                    start=(ko == 0),
                                    stop=(ko == KO - 1),
                                )
                        nc.vector.tensor_copy(
                            pair_t[:, c0:c1], ps[:, 0:c1 - c0],
                        )
                    qkv.append(pair_t)
                qT, kT, vm = qkv

                # V to natural layout via PE transpose, both halves at once;
                # vnat[:, j, g, 64] = 1 for real (non-pad) compacted keys
                vnat = proj.tile([128, nblkc, 2, DH + 1], bf16, tag="vnat",
                                 name=f"vnat{p}")
                for j in range(nblkc):
                    pst = psT.tile([128, 128], bf16, tag="pst",
                                   name=f"pst{p}{j}")
                    nc.tensor.transpose(
                        pst[:], vm[:, 128 * j:128 * (j + 1)], ident_bf[:]
                    )
                    nc.vector.tensor_copy(vnat[:, j, :, 0:DH], pst[:])
                nc.gpsimd.tensor_copy(
                    vnat[:, :, :, DH],
                    kmc_sb[:, :, None].to_broadcast((128, nblkc, 2)),
                )
                return qT, kT, vnat

            def emit_attn(p, tiles):
                qT, kT, vnat = tiles
                pos = {
                    g: psOT.tile([DH + 1, nq], f32, tag="psot",
                                 name=f"po{p}{g}")
                    for g in range(2)
                }

                def emit_panel(j, g):
                    """scores^T panel j + exp + visibility mask -> pt."""
                    gs = slice(64 * g, 64 * (g + 1))
                    c0 = 128 * sb[j]
                    chunks = _chunks_for(c0, nq)
                    pt = ptp.tile([128, nq - c0], bf16, tag="pt",
                                  name=f"pt{p}{g}{j}")
                    for (lo, hi) in chunks:
                        ss = psS.tile([128, 512], f32, tag="ps",
                                      name=f"ss{p}{g}{j}{lo}")
                        nc.tensor.matmul(
                            ss[:, 0:hi - lo],
                            lhsT=kT[gs, 128 * j:128 * (j + 1)],
                            rhs=qT[gs, c0 + lo:c0 + hi],
                            start=True,
                            stop=True,
                        )
                        nc.scalar.activation(
                            pt[:, lo:hi],
                            ss[:, 0:hi - lo],
                            mybir.ActivationFunctionType.Exp,
                            bias=0.0,
                            scale=1.0,
                        )
                        # visibility mask: causal (compacted coords) + pads;
                        # split across DVE/Pool
                        eng = nc.vector if (j + g) % 2 else nc.gpsimd
                        eng.tensor_tensor(
                            pt[:, lo:hi],
                            pt[:, lo:hi],
                            msk_sb[:, int(off[j]) + lo:int(off[j]) + hi],
                            mybir.AluOpType.mult,
                        )
                    return pt, chunks

                def emit_pv(j, g, pt, chunks):
                    # PV accumulate: po[:, q] += vnat_j^T @ pt
                    c0 = 128 * sb[j]
                    for (lo, hi) in chunks:
                        stop = (j == lb0 and c0 + lo < 512) or (
                            j == nblkc - 1 and c0 + lo >= 512
                        )
                        nc.tensor.matmul(
                            pos[g][:, c0 + lo:c0 + hi],
                            lhsT=vnat[:, j, g, :],
                            rhs=pt[:, lo:hi],
                            start=(j == 0),
                            stop=bool(stop),
                            skip_group_check=True,
                        )

                # g-interleaved panels with a one-step j pipeline
                prev = [emit_panel(0, 0), emit_panel(0, 1)]
                for j in range(1, nblkc):
                    cur = [emit_panel(j, 0), emit_panel(j, 1)]
                    for g in range(2):
                        emit_pv(j - 1, g, *prev[g])
                    prev = cur
                for g in range(2):
                    emit_pv(nblkc - 1, g, *prev[g])
                    nc.scalar.copy(sr_tiles[g][0:1, :],
                                   pos[g][DH:DH + 1, :])
                # tails: bcast(sum+flag), approx reciprocal, multiply
                for g in range(2):
                    gs = slice(64 * g, 64 * (g + 1))
                    po = pos[g]
                    for (c0, c1) in qch:
                        cols = slice(c0, c1)
                        bc = psS.tile([128, 512], f32, tag="ps",
                                      name=f"bc{p}{g}{c0}")
                        nc.tensor.matmul(
                            bc[0:DH, 0:c1 - c0],
                            lhsT=ones64[:, :],
                            rhs=sr_tiles[g][:, cols],
                            start=True,
                            stop=True,
                        )
                        rcpb = rowp.tile([DH, 512], f32, tag="bcs",
                                         name=f"rcpb{p}{g}{c0}")
                        nc.vector.reciprocal_approx_fast(
                            rcpb[:, 0:c1 - c0], bc[0:DH, 0:c1 - c0]
                        )
                        nc.vector.tensor_tensor(
                            ot_sb[p][gs, cols],
                            po[0:DH, cols],
                            rcpb[:, 0:c1 - c0],
                            mybir.AluOpType.mult,
                        )
                    nc.vector.tensor_tensor(
                        ot_sb[p][gs, 0:128],
                        ot_sb[p][gs, 0:128],
                        corr_sb[gs, p, :],
                        mybir.AluOpType.add,
                    )

            # ---- software-pipelined emission: proj one pair ahead ----
            tiles = emit_proj(0)
            nc.sync.dma_start(wo_sb[:], wo_d.rearrange("p ki n -> ki p n"))
            nc.sync.dma_start(corr_sb[:], corr_d[:, :, :])
            nxt = emit_proj(1)
            emit_attn(0, tiles)
            tiles, nxt = nxt, emit_proj(2)
            emit_attn(1, tiles)
            tiles, nxt = nxt, emit_proj(3)
            emit_attn(2, tiles)
            emit_attn(3, nxt)

            # ---- final projection + relu (qm already applied by
            # compaction: dropped rows are host-scattered zeros) ----
            for i in range(nqblk):
                ps = psS.tile([128, 512], f32, tag="ps", name=f"psf{i}")
                for p in range(NPAIR):
                    nc.tensor.matmul(
                        ps[:],
                        lhsT=ot_sb[p][:, 128 * i:128 * (i + 1)],
                        rhs=wo_sb[:, p, :],
                        start=(p == 0),
                        stop=(p == NPAIR - 1),
                    )
                o_sb = outp.tile([128, D], f32, tag="osb", name=f"osb{i}")
                nc.scalar.activation(
                    o_sb[:],
                    ps[:],
                    mybir.ActivationFunctionType.Relu,
                    bias=0.0,
                    scale=1.0,
                )
                nc.sync.dma_start(out_d[128 * i:128 * (i + 1), :], o_sb[:])

    nc.compile()
    return nc


def _get_nc(nkc, sb, nq):
    key = (nkc, tuple(sb), nq)
    if key not in _CACHE:
        _CACHE[key] = _build(nkc, sb, nq)
    return _CACHE[key]


def _host_prep(query, key, value, query_mask, key_mask, Wq, Wk, Wv, Wo):
    """Per-core input maps + shared compaction geometry."""
    inv = np.float32(1.0) / np.sqrt(np.float32(D))

    import ml_dtypes

    bfl = ml_dtypes.bfloat16

    def tfeat(x):  # (B,S,D) -> feature-major (D, B*S), contiguous bf16
        return np.ascontiguousarray(
            x.reshape(B * S, D).astype(np.float32, copy=False).T
        ).astype(bfl)

    xq, xk, xv = tfeat(query), tfeat(key), tfeat(value)
    kmf = key_mask.astype(np.float32)
    qmf = query_mask.astype(np.float32)
    Wqf = Wq.astype(np.float32, copy=False)
    Wkf = Wk.astype(np.float32, copy=False)
    Wvf = Wv.astype(np.float32, copy=False)
    Wof = Wo.astype(np.float32, copy=False)

    wo_p = np.stack(
        [
            np.concatenate(
                [Wof[p * DH:(p + 1) * DH, :], Wof[(p + 4) * DH:(p + 5) * DH, :]],
                axis=0,
            )
            for p in range(NPAIR)
        ]
    ).astype(bfl)  # (4, 128, 512)

    # ---- compaction geometry (shared across cores at build time) ----
    idxk = [np.nonzero(kmf[a])[0] for a in range(H)]
    idxq = [np.nonzero(qmf[a])[0] for a in range(H)]
    nkc = 128 * int(np.ceil(max(len(i) for i in idxk) / 128.0))
    nq = 128 * int(np.ceil(max(len(i) for i in idxq) / 128.0))
    nblkc = nkc // 128
    sb = []
    for jp in range(nblkc):
        starts = []
        for a in range(H):
            if len(idxk[a]) > 128 * jp:
                pos = int(idxk[a][128 * jp])
                starts.append(int(np.searchsorted(idxq[a], pos)))
        sb.append(min(starts) // 128 if starts else 0)
    assert sb[0] == 0, "first compacted key block must start at q block 0"
    pw = [nq - 128 * sbj for sbj in sb]
    off = np.concatenate([[0], np.cumsum(pw)]).astype(int)
    sumw = int(off[-1])

    in_maps = []
    for a in range(H):
        km = kmf[a]
        ik, iq = idxk[a], idxq[a]
        n_k, n_q = len(ik), len(iq)
        # flag rows: every visible key masked (faithful-TF uniform tie case)
        cs = np.cumsum(km)
        flg_full = (cs == 0).astype(np.float32)      # (S,), original coords
        corrT = np.zeros((128, NPAIR, 128), np.float32)
        kept_flag = [cq for cq in range(n_q) if flg_full[iq[cq]]]
        if kept_flag:
            assert max(kept_flag) < 128, "flag rows beyond block 0"
            wv_a = Wvf[:, a * DH:(a + 1) * DH]       # (512, 64)
            tail_cnt = km.sum()
            for p in range(NPAIR):
                for g in range(2):
                    c = p + 4 * g
                    vfull = value[c].astype(np.float32)      # (S, 512)
                    mtot = (km[:, None] * vfull).sum(axis=0)  # (512,)
                    for cq in kept_flag:
                        sq = int(iq[cq])
                        pre = vfull[:sq + 1].sum(axis=0)
                        count = (sq + 1) + tail_cnt
                        corrT[64 * g:64 * (g + 1), p, cq] = (
                            (pre + mtot) @ wv_a
                        ) / np.float32(count)

        # per-core compacted streams (pads stay zero)
        xq_c = np.zeros((D, B * nq), bfl)
        xk_c = np.zeros((D, B * nkc), bfl)
        xv_c = np.zeros((D, B * nkc), bfl)
        for c in range(B):
            xq_c[:, c * nq:c * nq + n_q] = xq[:, c * S + iq]
            xk_c[:, c * nkc:c * nkc + n_k] = xk[:, c * S + ik]
            xv_c[:, c * nkc:c * nkc + n_k] = xv[:, c * S + ik]

        # visibility mask in compacted coords: key r of block jp visible to
        # compacted query cq iff ik[...] <= iq[cq]; pads never visible
        maskc = np.zeros((128, sumw), np.float32)
        for jp in range(nblkc):
            base = 128 * sb[jp]
            for r in range(128):
                ki = 128 * jp + r
                if ki < n_k:
                    cq0 = int(np.searchsorted(iq, int(ik[ki])))
                    s0 = max(0, cq0 - base)
                    maskc[r, int(off[jp]) + s0:int(off[jp + 1])] = 1.0
        kmc = np.zeros((128, nblkc), np.float32)
        for jp in range(nblkc):
            kmc[:, jp] = (128 * jp + np.arange(128) < n_k)
        # flag indicator in compacted coords; pads flagged too (keeps the
        # reciprocal away from 0)
        flg_c = np.ones(nq, np.float32)
        flg_c[:n_q] = flg_full[iq]

        in_maps.append(
            {
                "xt_q": xq_c,
                "xt_k": xk_c,
                "xt_v": xv_c,
                "wq": np.ascontiguousarray(
                    Wqf[:, a * DH:(a + 1) * DH] * inv
                ).astype(bfl),
                "wk": np.ascontiguousarray(
                    Wkf[:, a * DH:(a + 1) * DH]
                ).astype(bfl),
                "wv": np.ascontiguousarray(
                    Wvf[:, a * DH:(a + 1) * DH]
                ).astype(bfl),
                "wo_p": wo_p,
                "maskc": maskc.astype(bfl),
                "kmc": kmc.astype(bfl),
                "flg": np.ascontiguousarray(flg_c[None, :]).astype(bfl),
                "corrT": corrT,
            }
        )
    return in_maps, nkc, sb, nq, idxq


def kernel(**inputs) -> np.ndarray:
    from concourse.bass_utils import run_bass_kernel_spmd

    in_maps, nkc, sb, nq, idxq = _host_prep(
        np.asarray(inputs["query"]),
        np.asarray(inputs["key"]),
        np.asarray(inputs["value"]),
        np.asarray(inputs["query_mask"]),
        np.asarray(inputs["key_mask"]),
        np.asarray(inputs["Wq"]),
        np.asarray(inputs["Wk"]),
        np.asarray(inputs["Wv"]),
        np.asarray(inputs["Wo"]),
    )
    nc = _get_nc(nkc, sb, nq)
    res = run_bass_kernel_spmd(nc, in_maps, core_ids=list(range(H)), **RUN_KWARGS)
    global LAST_RESULT
    LAST_RESULT = res
    full = np.zeros((H, S, D), np.float32)
    for a in range(H):
        n_q = len(idxq[a])
        full[a][idxq[a]] = res.results[a]["out"][:n_q]
    return full


# revision 58
# speedup vs baseline: 1.1031x; 1.1022x over previous
"""Trainium2 Bass kernel for nn_MultiHeadAttention_61778809586301.

Head-sharded across 8 NeuronCores: core `a` computes output row-group `a`
(per the reference's faithful-TF recombination: head `a` across all 8
batches, concatenated batch-major along channels, then Wo+relu).

Design:
  - scores computed TRANSPOSED (S^T = K^T blocks vs Q panels) so the P^T
    the PV matmul needs comes straight out of exp() — no P transposes.
  - no max-subtraction softmax; the PV matmul also emits the softmax
    denominator via an extra ones column; normalization happens on the
    small O^T via an fp32r broadcast + approx reciprocal.
  - KEY+QUERY COMPACTION: each core attends with one key_mask row and one
    query_mask row (faithful-TF quirk).  The host compacts away masked
    keys AND masked queries per core (~2x fewer of each), uploads
    per-core compacted xt_q/xt_k/xt_v and a compacted-coordinate
    visibility mask (causal+pad) applied after exp.  Panel bounds are
    fixed at build time from the min visible-query position across cores.
    Dropped query rows are exactly zero in the reference (relu * qm), so
    the host scatters device rows into a zeros array afterwards.
  - fully-masked-row semantics handled by host-precomputed corrections.
"""
import sys

if "/opt/trn_rl_repo" not in sys.path:
    sys.path.insert(0, "/opt/trn_rl_repo")

import numpy as np

B, S, D, H, DH = 8, 1024, 512, 8, 64
NPAIR = 4          # batch pairs (p, p+4)
KO = D // 128      # 4 contraction chunks of 128

_CACHE: dict = {}
RUN_KWARGS: dict = {}   # extra kwargs for run_bass_kernel_spmd (e.g. trace)
LAST_RESULT = None      # BassKernelResults of the most recent kernel() call


def _chunks_for(c0, nq):
    """Split compacted-q columns [c0, nq) at the 512 PSUM-bank boundary."""
    if c0 < 512 < nq:
        return [(0, 512 - c0), (512 - c0, nq - c0)]
    return [(0, nq - c0)]


def _build(nkc, sb, nq):
    import concourse.mybir as mybir
    import concourse.tile as tile
    from concourse import bacc
    from concourse.masks import make_identity

    nblkc = nkc // 128
    nqblk = nq // 128
    pw = [nq - 128 * sb[j] for j in range(nblkc)]   # panel widths
    off = np.concatenate([[0], np.cumsum(pw)]).astype(int)
    sumw = int(off[-1])
    # last panel whose chunks touch PSUM bank 0 (compacted-q col < 512)
    lb0 = max(j for j in range(nblkc) if 128 * sb[j] < 512)
    qch = _chunks_for(0, nq)    # [(0,512),(512,nq)] or [(0,nq)]

    f32 = mybir.dt.float32
    bf16 = mybir.dt.bfloat16
    nc = bacc.Bacc(
        "TRN2",
        target_bir_lowering=False,
        debug=False,
        enable_asserts=False,
        num_devices=H,
    )

    xt_q = nc.dram_tensor("xt_q", [D, B * nq], bf16, kind="ExternalInput")
    xt_k = nc.dram_tensor("xt_k", [D, B * nkc], bf16, kind="ExternalInput")
    xt_v = nc.dram_tensor("xt_v", [D, B * nkc], bf16, kind="ExternalInput")
    wq_d = nc.dram_tensor("wq", [D, DH], bf16, kind="ExternalInput")
    wk_d = nc.dram_tensor("wk", [D, DH], bf16, kind="ExternalInput")
    wv_d = nc.dram_tensor("wv", [D, DH], bf16, kind="ExternalInput")
    wo_d = nc.dram_tensor("wo_p", [NPAIR, 128, D], bf16, kind="ExternalInput")
    msk_d = nc.dram_tensor("maskc", [128, sumw], bf16, kind="ExternalInput")
    kmc_d = nc.dram_tensor("kmc", [128, nblkc], bf16, kind="ExternalInput")
    flg_d = nc.dram_tensor("flg", [1, nq], bf16, kind="ExternalInput")
    corr_d = nc.dram_tensor("corrT", [128, NPAIR, 128], f32,
                            kind="ExternalInput")
    out_d = nc.dram_tensor("out", [nq, D], f32, kind="ExternalOutput")

    kvch = [(0, min(512, nkc))] + ([(512, nkc)] if nkc > 512 else [])

    with tile.TileContext(nc) as tc:
        with (
            tc.tile_pool(name="fixed", bufs=1) as fixed,
            tc.tile_pool(name="stage", bufs=3) as stage,
            tc.tile_pool(name="proj", bufs=2) as proj,
            tc.tile_pool(name="ptp", bufs=5) as ptp,
            tc.tile_pool(name="rowp", bufs=2) as rowp,
            tc.tile_pool(name="outp", bufs=3) as outp,
            tc.tile_pool(name="psS", bufs=3, space="PSUM") as psS,
            tc.tile_pool(name="psOT", bufs=2, space="PSUM") as psOT,
            tc.tile_pool(name="psT", bufs=1, space="PSUM") as psT,
        ):
            # ---- constants / weights ----
            ident = fixed.tile([128, 128], f32, tag="ident")
            make_identity(nc, ident[:])
            ident_bf = fixed.tile([128, 128], bf16, tag="identbf")
            nc.gpsimd.tensor_copy(ident_bf[:], ident[:])

            wq_sb = fixed.tile([128, KO, DH], bf16, tag="wq")
            wk_sb = fixed.tile([128, KO, DH], bf16, tag="wk")
            wv_sb = fixed.tile([128, KO, DH], bf16, tag="wv")
            nc.scalar.dma_start(wq_sb[:], wq_d.rearrange("(ko ki) m -> ki ko m", ki=128))
            nc.scalar.dma_start(wk_sb[:], wk_d.rearrange("(ko ki) m -> ki ko m", ki=128))
            nc.scalar.dma_start(wv_sb[:], wv_d.rearrange("(ko ki) m -> ki ko m", ki=128))

            msk_sb = fixed.tile([128, sumw], bf16, tag="msk")
            nc.gpsimd.dma_start(msk_sb[:], msk_d[:, :])
            kmc_sb = fixed.tile([128, nblkc], bf16, tag="kmc")
            nc.gpsimd.dma_start(kmc_sb[:], kmc_d[:, :])
            flg_sb = fixed.tile([1, nq], bf16, tag="flg")
            nc.gpsimd.dma_start(flg_sb[:], flg_d[:, :])
            # wo/corr are consumed late; loads emitted after pair-0 staging
            wo_sb = fixed.tile([128, NPAIR, D], bf16, tag="wo")
            corr_sb = fixed.tile([128, NPAIR, 128], f32, tag="corr")

            # ones rows for the rcp broadcast (K=2: sum row + flag row)
            f32r = mybir.dt.float32r
            ones64f = fixed.tile([2, DH], f32, tag="ones64f")
            nc.vector.memset(ones64f[:], 1.0)
            ones64 = fixed.tile([2, DH], f32r, tag="ones64")
            nc.scalar.copy(ones64[:], ones64f[:])

            # persistent per-half sum-row tiles; row 1 preloaded with the
            # flag-row indicator so the broadcast matmul adds it for free
            flgr = fixed.tile([1, nq], f32r, tag="flgr")
            nc.scalar.copy(flgr[:], flg_sb[:])
            sr_tiles = []
            for gg in range(2):
                srt = fixed.tile([2, nq], f32r, tag=f"srt{gg}",
                                 name=f"srt{gg}")
                nc.scalar.dma_start(srt[1:2, :], flgr[0:1, :])
                sr_tiles.append(srt)

            # persistent attention outputs, transposed
            ot_sb = [
                fixed.tile([128, nq], bf16, tag=f"ot{p}", name=f"ot{p}")
                for p in range(NPAIR)
            ]

            def emit_proj(p):
                """q/k/v projections for pair p (all on compacted streams)."""
                qkv = []
                for name, xt, wid, ch in (
                    ("q", xt_q, nq, qch), ("k", xt_k, nkc, kvch),
                    ("v", xt_v, nkc, kvch),
                ):
                    pair_t = proj.tile([128, wid], bf16, tag=f"{name}T",
                                       name=f"{name}T{p}")
                    sts = []
                    for g in range(2):
                        c = p + 4 * g
                        st = stage.tile(
                            [128, KO, wid], bf16,
                            tag="xsq" if name == "q" else "xskv",
                            bufs=4 if name == "q" else 8,
                            name=f"st{p}{name}{g}",
                        )
                        nc.sync.dma_start(
                            st[:],
                            xt[:, c * wid:(c + 1) * wid].rearrange(
                                "(ko ki) s -> ki ko s", ki=128
                            ),
                        )
                        sts.append(st)
                    w_sb = {"q": wq_sb, "k": wk_sb, "v": wv_sb}[name]
                    for (c0, c1) in ch:
                        ps = psS.tile([128, 512], f32, tag="ps",
                                      name=f"psp{p}{name}{c0}")
                        for ko in range(KO):
                            for g in range(2):
                                nc.tensor.matmul(
                                    ps[64 * g:64 * (g + 1), 0:c1 - c0],
                                    lhsT=w_sb[:, ko, :],
                                    rhs=sts[g][:, ko, c0:c1],
                                    start=(ko == 0),
                                    stop=(ko == KO - 1),
                                )
                        nc.vector.tensor_copy(
                            pair_t[:, c0:c1], ps[:, 0:c1 - c0],
                        )
                    qkv.append(pair_t)
                qT, kT, vm = qkv

                # V to natural layout via PE transpose, both halves at once;
                # vnat[:, j, g, 64] = 1 for real (non-pad) compacted keys
                vnat = proj.tile([128, nblkc, 2, DH + 1], bf16, tag="vnat",
                                 name=f"vnat{p}")
                for j in range(nblkc):
                    pst = psT.tile([128, 128], bf16, tag="pst",
                                   name=f"pst{p}{j}")
                    nc.tensor.transpose(
                        pst[:], vm[:, 128 * j:128 * (j + 1)], ident_bf[:]
                    )
                    nc.vector.tensor_copy(vnat[:, j, :, 0:DH], pst[:])
                nc.gpsimd.tensor_copy(
                    vnat[:, :, :, DH],
                    kmc_sb[:, :, None].to_broadcast((128, nblkc, 2)),
                )
                return qT, kT, vnat

            def emit_attn_panels(p, tiles):
                qT, kT, vnat = tiles
                pos = {
                    g: psOT.tile([DH + 1, nq], f32, tag="psot",
                                 name=f"po{p}{g}")
                    for g in range(2)
                }

                def emit_panel(j, g):
                    """scores^T panel j + exp + visibility mask -> pt."""
                    gs = slice(64 * g, 64 * (g + 1))
                    c0 = 128 * sb[j]
                    chunks = _chunks_for(c0, nq)
                    pt = ptp.tile([128, nq - c0], bf16, tag="pt",
                                  name=f"pt{p}{g}{j}")
                    for (lo, hi) in chunks:
                        ss = psS.tile([128, 512], f32, tag="ps",
                                      name=f"ss{p}{g}{j}{lo}")
                        nc.tensor.matmul(
                            ss[:, 0:hi - lo],
                            lhsT=kT[gs, 128 * j:128 * (j + 1)],
                            rhs=qT[gs, c0 + lo:c0 + hi],
                            start=True,
                            stop=True,
                        )
                        nc.scalar.activation(
                            pt[:, lo:hi],
                            ss[:, 0:hi - lo],
                            mybir.ActivationFunctionType.Exp,
                            bias=0.0,
                            scale=1.0,
                        )
                        # visibility mask: causal (compacted coords) + pads;
                        # split across DVE/Pool
                        eng = nc.vector if (j + g) % 2 else nc.gpsimd
                        eng.tensor_tensor(
                            pt[:, lo:hi],
                            pt[:, lo:hi],
                            msk_sb[:, int(off[j]) + lo:int(off[j]) + hi],
                            mybir.AluOpType.mult,
                        )
                    return pt, chunks

                def emit_pv(j, g, pt, chunks):
                    # PV accumulate: po[:, q] += vnat_j^T @ pt
                    c0 = 128 * sb[j]
                    for (lo, hi) in chunks:
                        stop = (j == lb0 and c0 + lo < 512) or (
                            j == nblkc - 1 and c0 + lo >= 512
                        )
                        nc.tensor.matmul(
                            pos[g][:, c0 + lo:c0 + hi],
                            lhsT=vnat[:, j, g, :],
                            rhs=pt[:, lo:hi],
                            start=(j == 0),
                            stop=bool(stop),
                            skip_group_check=True,
                        )

                # g-interleaved panels with a one-step j pipeline
                prev = [emit_panel(0, 0), emit_panel(0, 1)]
                for j in range(1, nblkc):
                    cur = [emit_panel(j, 0), emit_panel(j, 1)]
                    for g in range(2):
                        emit_pv(j - 1, g, *prev[g])
                    prev = cur
                for g in range(2):
                    emit_pv(nblkc - 1, g, *prev[g])
                    nc.scalar.copy(sr_tiles[g][0:1, :],
                                   pos[g][DH:DH + 1, :])
                return pos

            def emit_attn_tails(p, pos):
                # tails: bcast(sum+flag), approx reciprocal, multiply;
                # emitted after the NEXT pair's proj so the bcast matmuls
                # never wait on the scalar sum-row copy
                for g in range(2):
                    gs = slice(64 * g, 64 * (g + 1))
                    po = pos[g]
                    for (c0, c1) in qch:
                        cols = slice(c0, c1)
                        bc = psS.tile([128, 512], f32, tag="ps",
                                      name=f"bc{p}{g}{c0}")
                        nc.tensor.matmul(
                            bc[0:DH, 0:c1 - c0],
                            lhsT=ones64[:, :],
                            rhs=sr_tiles[g][:, cols],
                            start=True,
                            stop=True,
                        )
                        rcpb = rowp.tile([DH, 512], f32, tag="bcs",
                                         name=f"rcpb{p}{g}{c0}")
                        nc.vector.reciprocal_approx_fast(
                            rcpb[:, 0:c1 - c0], bc[0:DH, 0:c1 - c0]
                        )
                        nc.vector.tensor_tensor(
                            ot_sb[p][gs, cols],
                            po[0:DH, cols],
                            rcpb[:, 0:c1 - c0],
                            mybir.AluOpType.mult,
                        )
                    nc.vector.tensor_tensor(
                        ot_sb[p][gs, 0:128],
                        ot_sb[p][gs, 0:128],
                        corr_sb[gs, p, :],
                        mybir.AluOpType.add,
                    )

            # ---- emission: panels(p) -> proj(p+1) -> tails(p): proj
            # never stalls on staging (it streams during panels(p)) and
            # the tails' PE work never stalls on the sum-row copy ----
            tiles = emit_proj(0)
            nc.sync.dma_start(wo_sb[:], wo_d.rearrange("p ki n -> ki p n"))
            nc.sync.dma_start(corr_sb[:], corr_d[:, :, :])
            pos = emit_attn_panels(0, tiles)
            for p in range(1, NPAIR):
                tiles = emit_proj(p)
                emit_attn_tails(p - 1, pos)
                pos = emit_attn_panels(p, tiles)
            emit_attn_tails(NPAIR - 1, pos)

            # ---- final projection + relu (qm already applied by
            # compaction: dropped rows are host-scattered zeros) ----
            for i in range(nqblk):
                ps = psS.tile([128, 512], f32, tag="ps", name=f"psf{i}")
                for p in range(NPAIR):
                    nc.tensor.matmul(
                        ps[:],
                        lhsT=ot_sb[p][:, 128 * i:128 * (i + 1)],
                        rhs=wo_sb[:, p, :],
                        start=(p == 0),
                        stop=(p == NPAIR - 1),
                    )
                o_sb = outp.tile([128, D], f32, tag="osb", name=f"osb{i}")
                nc.scalar.activation(
                    o_sb[:],
                    ps[:],
                    mybir.ActivationFunctionType.Relu,
                    bias=0.0,
                    scale=1.0,
                )
                nc.sync.dma_start(out_d[128 * i:128 * (i + 1), :], o_sb[:])

    nc.compile()
    return nc


def _get_nc(nkc, sb, nq):
    key = (nkc, tuple(sb), nq)
    if key not in _CACHE:
        _CACHE[key] = _build(nkc, sb, nq)
    return _CACHE[key]


def _host_prep(query, key, value, query_mask, key_mask, Wq, Wk, Wv, Wo):
    """Per-core input maps + shared compaction geometry."""
    inv = np.float32(1.0) / np.sqrt(np.float32(D))

    import ml_dtypes

    bfl = ml_dtypes.bfloat16

    def tfeat(x):  # (B,S,D) -> feature-major (D, B*S), contiguous bf16
        return np.ascontiguousarray(
            x.reshape(B * S, D).astype(np.float32, copy=False).T
        ).astype(bfl)

    xq, xk, xv = tfeat(query), tfeat(key), tfeat(value)
    kmf = key_mask.astype(np.float32)
    qmf = query_mask.astype(np.float32)
    Wqf = Wq.astype(np.float32, copy=False)
    Wkf = Wk.astype(np.float32, copy=False)
    Wvf = Wv.astype(np.float32, copy=False)
    Wof = Wo.astype(np.float32, copy=False)

    wo_p = np.stack(
        [
            np.concatenate(
                [Wof[p * DH:(p + 1) * DH, :], Wof[(p + 4) * DH:(p + 5) * DH, :]],
                axis=0,
            )
            for p in range(NPAIR)
        ]
    ).astype(bfl)  # (4, 128, 512)

    # ---- compaction geometry (shared across cores at build time) ----
    idxk = [np.nonzero(kmf[a])[0] for a in range(H)]
    idxq = [np.nonzero(qmf[a])[0] for a in range(H)]
    nkc = 128 * int(np.ceil(max(len(i) for i in idxk) / 128.0))
    nq = 128 * int(np.ceil(max(len(i) for i in idxq) / 128.0))
    nblkc = nkc // 128
    sb = []
    for jp in range(nblkc):
        starts = []
        for a in range(H):
            if len(idxk[a]) > 128 * jp:
                pos = int(idxk[a][128 * jp])
                starts.append(int(np.searchsorted(idxq[a], pos)))
        sb.append(min(starts) // 128 if starts else 0)
    assert sb[0] == 0, "first compacted key block must start at q block 0"
    pw = [nq - 128 * sbj for sbj in sb]
    off = np.concatenate([[0], np.cumsum(pw)]).astype(int)
    sumw = int(off[-1])

    in_maps = []
    for a in range(H):
        km = kmf[a]
        ik, iq = idxk[a], idxq[a]
        n_k, n_q = len(ik), len(iq)
        # flag rows: every visible key masked (faithful-TF uniform tie case)
        cs = np.cumsum(km)
        flg_full = (cs == 0).astype(np.float32)      # (S,), original coords
        corrT = np.zeros((128, NPAIR, 128), np.float32)
        kept_flag = [cq for cq in range(n_q) if flg_full[iq[cq]]]
        if kept_flag:
            assert max(kept_flag) < 128, "flag rows beyond block 0"
            wv_a = Wvf[:, a * DH:(a + 1) * DH]       # (512, 64)
            tail_cnt = km.sum()
            for p in range(NPAIR):
                for g in range(2):
                    c = p + 4 * g
                    vfull = value[c].astype(np.float32)      # (S, 512)
                    mtot = (km[:, None] * vfull).sum(axis=0)  # (512,)
                    for cq in kept_flag:
                        sq = int(iq[cq])
                        pre = vfull[:sq + 1].sum(axis=0)
                        count = (sq + 1) + tail_cnt
                        corrT[64 * g:64 * (g + 1), p, cq] = (
                            (pre + mtot) @ wv_a
                        ) / np.float32(count)

        # per-core compacted streams (pads stay zero)
        xq_c = np.zeros((D, B * nq), bfl)
        xk_c = np.zeros((D, B * nkc), bfl)
        xv_c = np.zeros((D, B * nkc), bfl)
        for c in range(B):
            xq_c[:, c * nq:c * nq + n_q] = xq[:, c * S + iq]
            xk_c[:, c * nkc:c * nkc + n_k] = xk[:, c * S + ik]
            xv_c[:, c * nkc:c * nkc + n_k] = xv[:, c * S + ik]

        # visibility mask in compacted coords: key r of block jp visible to
        # compacted query cq iff ik[...] <= iq[cq]; pads never visible
        maskc = np.zeros((128, sumw), np.float32)
        for jp in range(nblkc):
            base = 128 * sb[jp]
            for r in range(128):
                ki = 128 * jp + r
                if ki < n_k:
                    cq0 = int(np.searchsorted(iq, int(ik[ki])))
                    s0 = max(0, cq0 - base)
                    maskc[r, int(off[jp]) + s0:int(off[jp + 1])] = 1.0
        kmc = np.zeros((128, nblkc), np.float32)
        for jp in range(nblkc):
            kmc[:, jp] = (128 * jp + np.arange(128) < n_k)
        # flag indicator in compacted coords; pads flagged too (keeps the
        # reciprocal away from 0)
        flg_c = np.ones(nq, np.float32)
        flg_c[:n_q] = flg_full[iq]

        in_maps.append(
            {
                "xt_q": xq_c,
                "xt_k": xk_c,
                "xt_v": xv_c,
                "wq": np.ascontiguousarray(
                    Wqf[:, a * DH:(a + 1) * DH] * inv
                ).astype(bfl),
                "wk": np.ascontiguousarray(
                    Wkf[:, a * DH:(a + 1) * DH]
                ).astype(bfl),
                "wv": np.ascontiguousarray(
                    Wvf[:, a * DH:(a + 1) * DH]
                ).astype(bfl),
                "wo_p": wo_p,
                "maskc": maskc.astype(bfl),
                "kmc": kmc.astype(bfl),
                "flg": np.ascontiguousarray(flg_c[None, :]).astype(bfl),
                "corrT": corrT,
            }
        )
    return in_maps, nkc, sb, nq, idxq


def kernel(**inputs) -> np.ndarray:
    from concourse.bass_utils import run_bass_kernel_spmd

    in_maps, nkc, sb, nq, idxq = _host_prep(
        np.asarray(inputs["query"]),
        np.asarray(inputs["key"]),
        np.asarray(inputs["value"]),
        np.asarray(inputs["query_mask"]),
        np.asarray(inputs["key_mask"]),
        np.asarray(inputs["Wq"]),
        np.asarray(inputs["Wk"]),
        np.asarray(inputs["Wv"]),
        np.asarray(inputs["Wo"]),
    )
    nc = _get_nc(nkc, sb, nq)
    res = run_bass_kernel_spmd(nc, in_maps, core_ids=list(range(H)), **RUN_KWARGS)
    global LAST_RESULT
    LAST_RESULT = res
    full = np.zeros((H, S, D), np.float32)
    for a in range(H):
        n_q = len(idxq[a])
        full[a][idxq[a]] = res.results[a]["out"][:n_q]
    return full
